# revision 12
# baseline (speedup 1.0000x reference)
"""PhasorLayer TRN2 kernel: data-parallel over batch across 8 NeuronCores.

Math (per batch row m):
  u     = x @ [Wk|Wq|wsum]^T + [bk|bq|sum_bv]   (KQS gemm, N=129, fp32)
  align = 64 - 2*sum_p sin^2((pi/2)*(tanh(uk)-tanh(uq)))
  gain  = softplus(align/64 + 0.5);  s = align*gain/64
  w     = x @ Wv^T + bv                          (GEMM1, f32r)
  muw   = mean(w);  varw = mean(w^2) - muw^2     (ssq via bf16 ones-matmuls)
  inv   = rsqrt(s^2*varw + 1e-5);  a = s*inv;  c = a*muw
  out   = xr + a*(w @ Wo2^T) - c*w1              (GEMM2, f32r)
  where Wo2 = Wo * ln_g (cols), w1 = rowsum(Wo2), xr = x + ln_b@Wo^T + bo

Precision choices: the KQS/phase path is fp32 (align is centered near 0 and
a = s*rsqrt(s^2 var + eps) amplifies encoder noise ~300x on near-zero-
resonance rows); the two big GEMMs are f32r (fp32 bits at 1 cyc/row, the
same PE rate as bf16 -- bf16 operands there push mean elementwise rel-err
to ~2e-2). Weights are shipped pre-transposed in SBUF-ready layouts so
every DMA moves >=1KB-per-partition contiguous lines. build_nc(reps=N)
emits the body N times back-to-back for dispatch-amortized timing.
"""

import sys

sys.path.insert(0, "/opt/trn_rl_repo")

import math
import os
from contextlib import ExitStack

import ml_dtypes
import numpy as np

import concourse.bass as bass
import concourse.mybir as mybir
import concourse.tile as tile
from concourse.alu_op_type import AluOpType
from concourse.bass_utils import run_bass_kernel_spmd
from concourse.mybir import dt
from concourse.tile_cfg import (
    BassTileBranchHintPlaceholder,
    BassTileConditionalBlock,
    BassTileCriticalSection,
    BassTileLoopBlock,
    BassTileSwitchBlock,
    TileBranchInst,
)
from concourse.vector_clock import ScopedClock

B, D, P = 8192, 4096, 64
NCORES = 8
M = B // NCORES  # 1024 batch rows per core
MT = M // 128    # 8 m-tiles
KD = D // 128    # 32 dim tiles
NB = D // 512    # 8 n-blocks
PI = math.pi
EPS = 1e-5
F32 = dt.float32
BF16 = dt.bfloat16
MMDT = dt.float32r  # big-GEMM operand dtype: fp32 bits, 1 cyc/row on PE
NPBF16 = ml_dtypes.bfloat16
AF = mybir.ActivationFunctionType

_SKIP_SPLIT = (
    BassTileBranchHintPlaceholder,
    BassTileConditionalBlock,
    BassTileCriticalSection,
    BassTileLoopBlock,
    BassTileSwitchBlock,
    TileBranchInst,
)


class LegalTileContext(tile.TileContext):
    """TileContext legalized to <=1 semaphore wait per instruction.

    This container's walrus rejects instructions with >1 sync wait. Extra
    waits are peeled onto single-wait NoOps on the same engine.
    """

    def _lower_ordered_insts(self, ordered):
        for insts in ordered.values():
            out = []
            for inst in insts:
                si = getattr(inst, "sync_info", None)
                if (
                    si is not None
                    and len(si.on_wait) > 1
                    and not isinstance(inst, _SKIP_SPLIT)
                ):
                    waits = list(si.on_wait)
                    for w in waits[:-1]:
                        nop = mybir.InstNoOp(
                            name=self.nc.get_next_instruction_name(),
                            text_hint="wait_split",
                            bass_nofuse=True,
                            engine=inst.engine,
                            sync_info=mybir.SyncInfo(on_wait=[w], on_update=[]),
                        )
                        out.append(nop)
                    inst.sync_info = mybir.SyncInfo(
                        on_wait=[waits[-1]], on_update=list(si.on_update)
                    )
                out.append(inst)
            insts[:] = out
        super()._lower_ordered_insts(ordered)

    def _drain_and_barrier(self, tick_clock, wait_clock):
        drain_inst = self.nc.sync.drain()
        wait_clock.add_sem_waits(
            drain_inst.ins, ScopedClock({None: tick_clock.global_clock})
        )
        si = drain_inst.ins.sync_info
        if si is not None and len(si.on_wait) > 1:
            waits = list(si.on_wait)
            drain_inst.ins.sync_info = mybir.SyncInfo(
                on_wait=[waits[0]], on_update=list(si.on_update)
            )
            for w in waits[1:]:
                nop = self.nc.sync.nop(nofuse=True, hint="wait_split")
                nop.ins.sync_info = mybir.SyncInfo(on_wait=[w], on_update=[])
        self.nc.all_engine_barrier()
        assert self.sems is not None
        popped = self.nc._tile_sem_poison_stack.pop()
        assert popped is self._sem_poison
        self.nc.clear_and_free_semaphores(list(self.sems.allocated().values()))
        self.nc.all_engine_barrier()


def build_body(nc, tc, ctx, dram, rep):
    """Emit one full kernel execution under TileContext tc."""
    r = f"r{rep}_"
    (xt_d, wvb_d, wob_d, wkqsb_d, brow_d, bvr_d, w1m_d, xrm_d, out_d,
     ssq_dram, wt_dram) = dram

    sb_small = ctx.enter_context(tc.tile_pool(name=r + "small", bufs=1))

    ones_t = sb_small.tile((128, 1), BF16, name=r + "ones", tag=r + "ones")
    nc.vector.memset(ones_t[:], 1.0)
    half_t = sb_small.tile((128, 1), F32, name=r + "half", tag=r + "half")
    nc.vector.memset(half_t[:], 0.5)
    eps_t = sb_small.tile((128, 1), F32, name=r + "epsb", tag=r + "epsb")
    nc.vector.memset(eps_t[:], EPS)
    brow_t = sb_small.tile((128, 129), F32, name=r + "browt", tag=r + "browt")
    nc.gpsimd.dma_start(brow_t[:], brow_d[:, :])
    bvr_t = sb_small.tile((128, KD), F32, name=r + "bvrt", tag=r + "bvrt")
    nc.gpsimd.dma_start(bvr_t[:], bvr_d[:, :])

    def col_tile(nm):
        return sb_small.tile((128, MT), F32, name=r + nm, tag=r + nm)

    red_all = col_tile("red_all")
    align_all = col_tile("align_all")
    e1_all = col_tile("e1_all")
    gain_all = col_tile("gain_all")
    s2_all = col_tile("s2_all")
    mu_all = col_tile("mu_all")
    ssq_all = col_tile("ssq_all")
    musq_all = col_tile("musq_all")
    var_all = col_tile("var_all")
    s_all = col_tile("s_all")
    s_sq_all = col_tile("s_sq_all")
    q_all = col_tile("q_all")
    q2_all = col_tile("q2_all")
    inv_all = col_tile("inv_all")
    a_all = col_tile("a_all")
    c_all = col_tile("c_all")
    cneg_all = col_tile("cneg_all")
    acc_sb = sb_small.tile((1, M), F32, name=r + "acc_sb", tag=r + "acc_sb")


    # ---------------- phase 1: KQS + GEMM1 (xt resident) ----------------
    with ExitStack() as p1:
        sb_xt = p1.enter_context(tc.tile_pool(name=r + "xtp", bufs=1))
        sb_s1 = p1.enter_context(tc.tile_pool(name=r + "s1", bufs=2))

        # f32r x^T residents for GEMM1, produced on-chip from the fp32 KQS
        # stream. The phase path needs full fp32 accuracy (align is centered
        # near 0 and a = s*rsqrt(s^2 var + eps) amplifies encoder noise
        # ~300x on near-zero-resonance rows), and the big GEMMs stay f32r
        # because the output rides on w@Wo2: bf16 operands there push the
        # mean elementwise rel-err to ~2e-2, the gate.
        # f32r x^T residents for GEMM1, produced on-chip (ACT copy performs
        # the f32r rounding) from the fp32 KQS stream -- the BIR verifier
        # requires f32r matmul operands to be explicitly rounded, so a
        # bitcast of the fp32 tiles is not legal here.
        xt_res = [
            sb_xt.tile((128, M), MMDT, name=f"{r}xt{j}", tag=f"{r}xt{j}")
            for j in range(KD)
        ]

        def xt_j(j):
            return xt_res[j][:]

        WKC = 8 * 129  # wkqs chunk: 8 j-tiles
        wkq_tiles = [None] * 4

        def load_wkq_chunk(c):
            t = sb_s1.tile((128, WKC), F32, name=r + "wkqc", tag=r + "wkqc", bufs=2)
            nc.scalar.dma_start(t[:], wkqsb_d[:, c * WKC : (c + 1) * WKC])
            wkq_tiles[c] = t

        load_wkq_chunk(0)

        # KQS gemm in fp32: stationary = x^T m-slice, moving = wkqs[j]
        with ExitStack() as pk:
            ps_kq = pk.enter_context(tc.tile_pool(name=r + "pskq", bufs=1, space="PSUM"))
            kq_pair = [
                ps_kq.tile((128, 258), F32, name=f"{r}kqp{i}", tag=f"{r}kqp{i}")
                for i in range(MT // 2)
            ]
            kq_list = [
                kq_pair[t // 2][:, (t % 2) * 129 : (t % 2) * 129 + 129]
                for t in range(MT)
            ]
            for j in range(KD):
                if j in (1, 9, 17):
                    load_wkq_chunk(j // 8 + 1)
                xs_t = sb_s1.tile((128, M), F32, name=r + "xs_t", tag=r + "xs", bufs=4)
                xs_eng = (nc.sync, nc.scalar, nc.gpsimd)[j % 3]
                xs_eng.dma_start(xs_t[:], xt_d[j * 128 : (j + 1) * 128, :])
                nc.scalar.activation(xt_res[j][:], xs_t[:], AF.Copy)
                for t in range(MT):
                    # two m-tiles share one PSUM bank => one accumulation
                    # group: start zeroes the bank at the first sub-tile,
                    # stop closes it at the last
                    nc.tensor.matmul(
                        kq_list[t],
                        xs_t[:, t * 128 : (t + 1) * 128],
                        wkq_tiles[j // 8][:, (j % 8) * 129 : (j % 8 + 1) * 129],
                        start=(j == 0 and t % 2 == 0),
                        stop=(j == KD - 1 and t % 2 == 1),
                    )
            # free all kq PSUM banks promptly: copy u = kq + brow to SBUF
            u_ts = []
            for t in range(MT):
                u_t = sb_s1.tile((128, 129), F32, name=r + "u_t", tag=r + "u", bufs=7)
                nc.vector.tensor_add(u_t[:], kq_list[t], brow_t[:])
                u_ts.append(u_t)

        # per-m-tile phase epilogue (ACT/DVE; overlaps GEMM1 matmuls on PE)
        for t in range(MT):
            u_t = u_ts[t]
            th_t = sb_s1.tile((128, 128), F32, name=r + "th_t", tag=r + "th")
            nc.scalar.activation(th_t[:], u_t[:, 0:128], AF.Tanh)
            d_t = sb_s1.tile((128, 64), F32, name=r + "d_t", tag=r + "d")
            nc.vector.tensor_sub(d_t[:], th_t[:, 0:64], th_t[:, 64:128])
            sn_t = sb_s1.tile((128, 64), F32, name=r + "sn_t", tag=r + "sn")
            nc.scalar.activation(sn_t[:], d_t[:], AF.Sin, scale=PI / 2)
            sq_t = sb_s1.tile((128, 64), F32, name=r + "sq_t", tag=r + "snsq")
            nc.scalar.activation(
                sq_t[:], sn_t[:], AF.Square, accum_out=red_all[:, t : t + 1]
            )
            nc.vector.tensor_scalar(
                align_all[:, t : t + 1],
                red_all[:, t : t + 1],
                -2.0,
                float(P),
                AluOpType.mult,
                AluOpType.add,
            )
            nc.scalar.activation(
                e1_all[:, t : t + 1],
                align_all[:, t : t + 1],
                AF.Exp,
                bias=half_t[:],
                scale=1.0 / P,
            )
            nc.scalar.activation(
                gain_all[:, t : t + 1], e1_all[:, t : t + 1], AF.Ln, bias=1.0
            )
            nc.vector.tensor_mul(
                s2_all[:, t : t + 1],
                align_all[:, t : t + 1],
                gain_all[:, t : t + 1],
            )
            nc.scalar.activation(
                mu_all[:, t : t + 1], u_t[:, 128:129], AF.Copy, scale=1.0 / D
            )

        # GEMM1: w^T tile kd = sum_j Wv^T[j, kd]^T @ x^T[j]  -> SBUF bf16
        # ssq = sum_k w^2 via ones-stationary matmuls, staggered one kd
        # behind the main stream so the PE never waits on sqw.
        ps_v = p1.enter_context(tc.tile_pool(name=r + "psv", bufs=2, space="PSUM"))
        ps_acc = p1.enter_context(tc.tile_pool(name=r + "psacc", bufs=1, space="PSUM"))
        acc_ps0 = ps_acc.tile((1, 512), F32, name=r + "acc_ps0", tag=r + "acc0")
        acc_ps1 = ps_acc.tile((1, 512), F32, name=r + "acc_ps1", tag=r + "acc1")

        sqw_tiles = [None] * KD

        def emit_ssq(kd):
            sqw_t = sqw_tiles[kd]
            nc.tensor.matmul(
                acc_ps0[:], ones_t[:], sqw_t[:, 0:512],
                start=(kd == 0), stop=(kd == KD - 1),
            )
            nc.tensor.matmul(
                acc_ps1[:], ones_t[:], sqw_t[:, 512:1024],
                start=(kd == 0), stop=(kd == KD - 1),
            )

        HD = D // 2
        for kd in range(KD):
            wv_h = []
            for hc in range(2):
                t = sb_s1.tile((128, HD), MMDT, name=r + "wv_t", tag=r + "wv", bufs=3)
                wv_eng = (nc.sync, nc.scalar, nc.gpsimd)[(2 * kd + hc) % 3]
                wv_eng.dma_start(
                    t[:], wvb_d[:, kd * D + hc * HD : kd * D + (hc + 1) * HD]
                )
                wv_h.append(t)
            v_ps = [
                ps_v.tile((128, 512), F32, name=f"{r}v_ps{h}", tag=f"{r}vps{h}")
                for h in range(2)
            ]
            for b in range(KD):
                wv_sl = wv_h[b // 16][:, (b % 16) * 128 : (b % 16 + 1) * 128]
                for h in range(2):
                    nc.tensor.matmul(
                        v_ps[h][:],
                        wv_sl,
                        xt_j(b)[:, h * 512 : (h + 1) * 512],
                        start=(b == 0),
                        stop=(b == KD - 1),
                    )
            if kd >= 1:
                emit_ssq(kd - 1)
            # bias add into fp32 tile; bounce to DRAM for the phase-2
            # residents; bf16 square for the (noise-tolerant) ssq reduction
            wtile = sb_s1.tile((128, M), F32, name=r + "wtile", tag=r + "wtile", bufs=3)
            for h in range(2):
                nc.vector.tensor_scalar(
                    wtile[:, h * 512 : (h + 1) * 512],
                    v_ps[h][:],
                    bvr_t[:, kd : kd + 1],
                    None,
                    AluOpType.add,
                )
            sqw_t = sb_s1.tile((128, M), BF16, name=r + "sqw_t", tag=r + "sqw", bufs=2)
            nc.scalar.activation(sqw_t[:], wtile[:], AF.Square)
            sqw_tiles[kd] = sqw_t
            wb_eng = (nc.gpsimd, nc.sync)[kd % 2]
            wb_eng.dma_start(wt_dram[kd, :, :], wtile[:].bitcast(MMDT))
        emit_ssq(KD - 1)

        # ssq bounce: [1, M] -> DRAM -> [128, MT] columns
        nc.scalar.copy(acc_sb[:, 0:512], acc_ps0[:])
        nc.scalar.copy(acc_sb[:, 512:1024], acc_ps1[:])
        nc.sync.dma_start(ssq_dram[:, :], acc_sb[:])
        for t in range(MT):
            nc.sync.dma_start(
                ssq_all[:, t : t + 1],
                ssq_dram[0:1, t * 128 : (t + 1) * 128].transpose([1, 0]),
            )

    # ---------------- scalar finalize ----------------
    nc.scalar.activation(musq_all[:], mu_all[:], AF.Square)
    nc.vector.tensor_scalar(var_all[:], ssq_all[:], 1.0 / D, None, AluOpType.mult)
    nc.vector.tensor_sub(var_all[:], var_all[:], musq_all[:])
    nc.scalar.activation(s_all[:], s2_all[:], AF.Copy, scale=1.0 / P)
    nc.scalar.activation(s_sq_all[:], s_all[:], AF.Square)
    nc.vector.tensor_mul(q_all[:], var_all[:], s_sq_all[:])
    nc.scalar.activation(q2_all[:], q_all[:], AF.Sqrt, bias=eps_t[:])
    nc.vector.reciprocal(inv_all[:], q2_all[:])
    nc.vector.tensor_mul(a_all[:], s_all[:], inv_all[:])
    nc.vector.tensor_mul(c_all[:], a_all[:], mu_all[:])
    nc.vector.tensor_scalar(cneg_all[:], c_all[:], -1.0, None, AluOpType.mult)

    # ---------------- phase 2: GEMM2 + epilogue (wt resident) ----------------
    with ExitStack() as p2:
        sb_wt = p2.enter_context(tc.tile_pool(name=r + "wtp", bufs=1))
        sb_s2 = p2.enter_context(tc.tile_pool(name=r + "s2", bufs=2))
        ps_p = p2.enter_context(tc.tile_pool(name=r + "psp", bufs=1, space="PSUM"))

        QN = 4 * 512  # wo chunk: 4 kd-slices of one nb
        NQ = KD // 4  # chunks per nb

        def load_wo_chunk(nb, q, eng):
            t = sb_s2.tile((128, QN), MMDT, name=r + "wo_q", tag=r + "woq", bufs=3)
            base = nb * (KD * 512)
            eng.dma_start(t[:], wob_d[:, base + q * QN : base + (q + 1) * QN])
            return t

        wo_first = load_wo_chunk(0, 0, nc.scalar)

        wt_res = []
        for k in range(KD):
            t = sb_wt.tile((128, M), MMDT, name=f"{r}wtr{k}", tag=f"{r}wtr{k}")
            eng = (nc.gpsimd, nc.sync)[k % 2]
            eng.dma_start(t[:], wt_dram[k, :, :])
            wt_res.append(t)

        w1_res = sb_s2.tile((128, D), F32, name=r + "w1_res", tag=r + "w1_res", bufs=1)
        nc.sync.dma_start(w1_res[:], w1m_d[:, :])

        def epilogue(nb, mt, p_tile):
            nsl = slice(nb * 512, (nb + 1) * 512)
            msl = slice(mt * 128, (mt + 1) * 128)
            xe_t = sb_s2.tile((128, 512), F32, name=r + "xe_t", tag=r + "xe", bufs=3)
            nc.gpsimd.dma_start(xe_t[:], xrm_d[msl, nsl])
            t1_t = sb_s2.tile((128, 512), F32, name=r + "t1_t", tag=r + "t1", bufs=3)
            nc.vector.scalar_tensor_tensor(
                t1_t[:], p_tile[:], a_all[:, mt : mt + 1], xe_t[:],
                AluOpType.mult, AluOpType.add,
            )
            oe_t = sb_s2.tile((128, 512), F32, name=r + "oe_t", tag=r + "oe", bufs=4)
            nc.vector.scalar_tensor_tensor(
                oe_t[:], w1_res[:, nsl], cneg_all[:, mt : mt + 1], t1_t[:],
                AluOpType.mult, AluOpType.add,
            )
            nc.sync.dma_start(out_d[msl, nsl], oe_t[:])

        for nb in range(NB):
            wo_q = []
            for q in range(NQ):
                if nb == 0 and q == 0:
                    wo_q.append(wo_first)
                    continue
                wo_q.append(load_wo_chunk(nb, q, nc.scalar))

            p_tiles = [
                ps_p.tile((128, 512), F32, name=f"{r}pp{mt}", tag=f"{r}pp{mt}")
                for mt in range(MT)
            ]
            for half in range(2):
                mts = range(half * 4, half * 4 + 4)
                for kd in range(KD):
                    wo_sl = wo_q[kd // 4][:, (kd % 4) * 512 : (kd % 4 + 1) * 512]
                    for mt in mts:
                        nc.tensor.matmul(
                            p_tiles[mt][:],
                            wt_res[kd][:, mt * 128 : (mt + 1) * 128],
                            wo_sl,
                            start=(kd == 0),
                            stop=(kd == KD - 1),
                        )
                for mt in mts:
                    epilogue(nb, mt, p_tiles[mt])


def build_nc(reps=1):
    nc = bass.Bass()
    xt_d = nc.declare_dram_parameter("xt", [D, M], F32, isOutput=False)
    wvb_d = nc.declare_dram_parameter("wvb", [128, KD * D], MMDT, isOutput=False)
    wob_d = nc.declare_dram_parameter("wob", [128, NB * KD * 512], MMDT, isOutput=False)
    wkqsb_d = nc.declare_dram_parameter("wkqsb", [128, KD * 129], F32, isOutput=False)
    brow_d = nc.declare_dram_parameter("brow", [128, 129], F32, isOutput=False)
    bvr_d = nc.declare_dram_parameter("bvr", [128, KD], F32, isOutput=False)
    w1m_d = nc.declare_dram_parameter("w1m", [128, D], F32, isOutput=False)
    xrm_d = nc.declare_dram_parameter("xrm", [M, D], F32, isOutput=False)
    out_d = nc.declare_dram_parameter("out", [M, D], F32, isOutput=True)
    ssq_dram = nc.dram_tensor("ssq_scr", [1, M], F32)
    wt_dram = nc.dram_tensor("wt_scr", [KD, 128, M], MMDT)
    dram = (xt_d, wvb_d, wob_d, wkqsb_d, brow_d, bvr_d, w1m_d, xrm_d,
            out_d, ssq_dram, wt_dram)

    for rep in range(reps):
        with ExitStack() as ctx:
            tc = ctx.enter_context(LegalTileContext(nc))
            build_body(nc, tc, ctx, dram, rep)
    return nc


def prep_host(inputs):
    """Host-side weight layout prep shared by all cores."""
    Wk = np.asarray(inputs["Wk"], dtype=np.float32)
    bk = np.asarray(inputs["bk"], dtype=np.float32)
    Wq = np.asarray(inputs["Wq"], dtype=np.float32)
    bq = np.asarray(inputs["bq"], dtype=np.float32)
    Wv = np.asarray(inputs["Wv"], dtype=np.float32)
    bv = np.asarray(inputs["bv"], dtype=np.float32)
    ln_g = np.asarray(inputs["ln_g"], dtype=np.float32)
    ln_b = np.asarray(inputs["ln_b"], dtype=np.float32)
    Wo = np.asarray(inputs["Wo"], dtype=np.float32)
    bo = np.asarray(inputs["bo"], dtype=np.float32)

    Wo2T = np.ascontiguousarray((Wo * ln_g[None, :]).T)  # [k, n]
    w1 = Wo2T.sum(axis=0)  # [n]
    rrow = (ln_b @ Wo.T + bo).astype(np.float32)  # [n]
    wsum = Wv.sum(axis=0)  # [j]
    wkqs = np.concatenate([Wk.T, Wq.T, wsum[:, None]], axis=1).astype(np.float32)
    brow = np.concatenate([bk, bq, [bv.sum()]]).astype(np.float32)

    # [128, KD*D]: wvb[p, kd*D + b*128 + kk] = Wv[kd*128+kk, b*128+p]
    wvb = np.ascontiguousarray(
        Wv.reshape(KD, 128, KD, 128).transpose(3, 0, 2, 1).reshape(128, KD * D)
    )
    # [128, NB*KD*512]: wob[p, nb*KD*512 + kd*512 + n'] = Wo2T[kd*128+p, nb*512+n']
    wob = np.ascontiguousarray(
        Wo2T.reshape(KD, 128, NB, 512).transpose(1, 2, 0, 3).reshape(128, NB * KD * 512)
    )
    # [128, KD*129]: wkqsb[p, j*129+c] = wkqs[j*128+p, c]  (fp32)
    wkqsb = np.ascontiguousarray(
        wkqs.reshape(KD, 128, 129).transpose(1, 0, 2).reshape(128, KD * 129)
    )
    brow_mat = np.ascontiguousarray(np.broadcast_to(brow, (128, 129)))
    bvr = np.ascontiguousarray(bv.reshape(KD, 128).T)  # [128, KD]
    w1m = np.ascontiguousarray(np.broadcast_to(w1, (128, D)), dtype=np.float32)
    return wvb, wob, wkqsb, brow_mat, bvr, w1m, rrow


def make_in_maps(x, prep):
    wvb, wob, wkqsb, brow_mat, bvr, w1m, rrow = prep
    in_maps = []
    for c in range(NCORES):
        xc = np.ascontiguousarray(x[c * M : (c + 1) * M])
        xtc = np.ascontiguousarray(xc.T)
        xrm = np.ascontiguousarray(xc + rrow[None, :])
        in_maps.append(
            {
                "xt": xtc,
                "wvb": wvb,
                "wob": wob,
                "wkqsb": wkqsb,
                "brow": brow_mat,
                "bvr": bvr,
                "w1m": w1m,
                "xrm": xrm,
            }
        )
    return in_maps


def kernel(**inputs):
    x = np.asarray(inputs["x"], dtype=np.float32)
    prep = prep_host(inputs)
    nc = build_nc(reps=1)
    in_maps = make_in_maps(x, prep)
    global LAST_BUILD
    LAST_BUILD = (nc, in_maps)
    res = run_bass_kernel_spmd(nc, in_maps, core_ids=list(range(NCORES)))
    global LAST_EXEC_NS
    LAST_EXEC_NS = getattr(res, "exec_time_ns", None)
    out = np.concatenate([res.results[c]["out"] for c in range(NCORES)], axis=0)
    return out.astype(np.float32)


LAST_EXEC_NS = None
LAST_BUILD = None


if __name__ == "__main__":
    rng = np.random.default_rng(0)
    ins = {
        "x": rng.standard_normal((B, D), dtype=np.float32),
        "Wk": rng.standard_normal((P, D), dtype=np.float32) / math.sqrt(D),
        "bk": np.zeros(P, np.float32),
        "Wq": rng.standard_normal((P, D), dtype=np.float32) / math.sqrt(D),
        "bq": np.zeros(P, np.float32),
        "Wv": rng.standard_normal((D, D), dtype=np.float32) / math.sqrt(D),
        "bv": np.zeros(D, np.float32),
        "ln_g": np.ones(D, np.float32),
        "ln_b": np.zeros(D, np.float32),
        "Wo": rng.standard_normal((D, D), dtype=np.float32) / math.sqrt(D),
        "bo": np.zeros(D, np.float32),
    }
    out = kernel(**ins)
    print("out", out.shape, out.dtype, float(np.abs(out).mean()))


# revision 13
# speedup vs baseline: 1.0808x; 1.0808x over previous
"""PhasorLayer TRN2 kernel: data-parallel over batch across 8 NeuronCores.

Math (per batch row m):
  u     = x @ [Wk|Wq|wsum]^T + [bk|bq|sum_bv]   (KQS gemm, N=129, fp32)
  align = 64 - 2*sum_p sin^2((pi/2)*(tanh(uk)-tanh(uq)))
  gain  = softplus(align/64 + 0.5);  s = align*gain/64
  w     = x @ Wv^T + bv                          (GEMM1, f32r)
  muw   = mean(w);  varw = mean(w^2) - muw^2     (ssq via bf16 ones-matmuls)
  inv   = rsqrt(s^2*varw + 1e-5);  a = s*inv;  c = a*muw
  out   = xr + a*(w @ Wo2^T) - c*w1              (GEMM2, f32r)
  where Wo2 = Wo * ln_g (cols), w1 = rowsum(Wo2), xr = x + ln_b@Wo^T + bo

Precision choices: the KQS/phase path is fp32 (align is centered near 0 and
a = s*rsqrt(s^2 var + eps) amplifies encoder noise ~300x on near-zero-
resonance rows); the two big GEMMs are f32r (fp32 bits at 1 cyc/row, the
same PE rate as bf16 -- bf16 operands there push mean elementwise rel-err
to ~2e-2). Weights are shipped pre-transposed in SBUF-ready layouts so
every DMA moves >=1KB-per-partition contiguous lines. build_nc(reps=N)
emits the body N times back-to-back for dispatch-amortized timing.
"""

import sys

sys.path.insert(0, "/opt/trn_rl_repo")

import math
import os
from contextlib import ExitStack

import ml_dtypes
import numpy as np

import concourse.bass as bass
import concourse.mybir as mybir
import concourse.tile as tile
from concourse.alu_op_type import AluOpType
from concourse.bass_utils import run_bass_kernel_spmd
from concourse.mybir import dt
from concourse.tile_cfg import (
    BassTileBranchHintPlaceholder,
    BassTileConditionalBlock,
    BassTileCriticalSection,
    BassTileLoopBlock,
    BassTileSwitchBlock,
    TileBranchInst,
)
from concourse.vector_clock import ScopedClock

B, D, P = 8192, 4096, 64
NCORES = 8
M = B // NCORES  # 1024 batch rows per core
MT = M // 128    # 8 m-tiles
KD = D // 128    # 32 dim tiles
NB = D // 512    # 8 n-blocks
PI = math.pi
EPS = 1e-5
F32 = dt.float32
BF16 = dt.bfloat16
MMDT = dt.float32r  # big-GEMM operand dtype: fp32 bits, 1 cyc/row on PE
NPBF16 = ml_dtypes.bfloat16
AF = mybir.ActivationFunctionType

_SKIP_SPLIT = (
    BassTileBranchHintPlaceholder,
    BassTileConditionalBlock,
    BassTileCriticalSection,
    BassTileLoopBlock,
    BassTileSwitchBlock,
    TileBranchInst,
)


class LegalTileContext(tile.TileContext):
    """TileContext legalized to <=1 semaphore wait per instruction.

    This container's walrus rejects instructions with >1 sync wait. Extra
    waits are peeled onto single-wait NoOps on the same engine.
    """

    def _lower_ordered_insts(self, ordered):
        for insts in ordered.values():
            out = []
            for inst in insts:
                si = getattr(inst, "sync_info", None)
                if (
                    si is not None
                    and len(si.on_wait) > 1
                    and not isinstance(inst, _SKIP_SPLIT)
                ):
                    waits = list(si.on_wait)
                    for w in waits[:-1]:
                        nop = mybir.InstNoOp(
                            name=self.nc.get_next_instruction_name(),
                            text_hint="wait_split",
                            bass_nofuse=True,
                            engine=inst.engine,
                            sync_info=mybir.SyncInfo(on_wait=[w], on_update=[]),
                        )
                        out.append(nop)
                    inst.sync_info = mybir.SyncInfo(
                        on_wait=[waits[-1]], on_update=list(si.on_update)
                    )
                out.append(inst)
            insts[:] = out
        super()._lower_ordered_insts(ordered)

    def _drain_and_barrier(self, tick_clock, wait_clock):
        drain_inst = self.nc.sync.drain()
        wait_clock.add_sem_waits(
            drain_inst.ins, ScopedClock({None: tick_clock.global_clock})
        )
        si = drain_inst.ins.sync_info
        if si is not None and len(si.on_wait) > 1:
            waits = list(si.on_wait)
            drain_inst.ins.sync_info = mybir.SyncInfo(
                on_wait=[waits[0]], on_update=list(si.on_update)
            )
            for w in waits[1:]:
                nop = self.nc.sync.nop(nofuse=True, hint="wait_split")
                nop.ins.sync_info = mybir.SyncInfo(on_wait=[w], on_update=[])
        self.nc.all_engine_barrier()
        assert self.sems is not None
        popped = self.nc._tile_sem_poison_stack.pop()
        assert popped is self._sem_poison
        self.nc.clear_and_free_semaphores(list(self.sems.allocated().values()))
        self.nc.all_engine_barrier()


def build_body(nc, tc, ctx, dram, rep):
    """Emit one full kernel execution under TileContext tc."""
    r = f"r{rep}_"
    (xt_d, wvb_d, wob_d, wkqsb_d, brow_d, bvr_d, w1m_d, xrm_d, out_d,
     ssq_dram, wt_dram) = dram

    sb_small = ctx.enter_context(tc.tile_pool(name=r + "small", bufs=1))

    ones_t = sb_small.tile((128, 1), BF16, name=r + "ones", tag=r + "ones")
    nc.vector.memset(ones_t[:], 1.0)
    half_t = sb_small.tile((128, 1), F32, name=r + "half", tag=r + "half")
    nc.vector.memset(half_t[:], 0.5)
    eps_t = sb_small.tile((128, 1), F32, name=r + "epsb", tag=r + "epsb")
    nc.vector.memset(eps_t[:], EPS)
    brow_t = sb_small.tile((128, 129), F32, name=r + "browt", tag=r + "browt")
    nc.gpsimd.dma_start(brow_t[:], brow_d[:, :])
    bvr_t = sb_small.tile((128, KD), F32, name=r + "bvrt", tag=r + "bvrt")
    nc.gpsimd.dma_start(bvr_t[:], bvr_d[:, :])

    def col_tile(nm):
        return sb_small.tile((128, MT), F32, name=r + nm, tag=r + nm)

    red_all = col_tile("red_all")
    align_all = col_tile("align_all")
    e1_all = col_tile("e1_all")
    gain_all = col_tile("gain_all")
    s2_all = col_tile("s2_all")
    mu_all = col_tile("mu_all")
    ssq_all = col_tile("ssq_all")
    musq_all = col_tile("musq_all")
    var_all = col_tile("var_all")
    s_all = col_tile("s_all")
    s_sq_all = col_tile("s_sq_all")
    q_all = col_tile("q_all")
    q2_all = col_tile("q2_all")
    inv_all = col_tile("inv_all")
    a_all = col_tile("a_all")
    c_all = col_tile("c_all")
    cneg_all = col_tile("cneg_all")
    acc_sb = sb_small.tile((1, M), F32, name=r + "acc_sb", tag=r + "acc_sb")


    # ---------------- phase 1: KQS + GEMM1 (xt resident) ----------------
    with ExitStack() as p1:
        sb_xt = p1.enter_context(tc.tile_pool(name=r + "xtp", bufs=1))
        sb_s1 = p1.enter_context(tc.tile_pool(name=r + "s1", bufs=2))

        # f32r x^T residents for GEMM1, produced on-chip from the fp32 KQS
        # stream. The phase path needs full fp32 accuracy (align is centered
        # near 0 and a = s*rsqrt(s^2 var + eps) amplifies encoder noise
        # ~300x on near-zero-resonance rows), and the big GEMMs stay f32r
        # because the output rides on w@Wo2: bf16 operands there push the
        # mean elementwise rel-err to ~2e-2, the gate.
        # f32r x^T residents for GEMM1, produced on-chip (ACT copy performs
        # the f32r rounding) from the fp32 KQS stream -- the BIR verifier
        # requires f32r matmul operands to be explicitly rounded, so a
        # bitcast of the fp32 tiles is not legal here.
        xt_res = [
            sb_xt.tile((128, M), MMDT, name=f"{r}xt{j}", tag=f"{r}xt{j}")
            for j in range(KD)
        ]

        def xt_j(j):
            return xt_res[j][:]

        WKC = 8 * 129  # wkqs chunk: 8 j-tiles
        wkq_tiles = [None] * 4

        def load_wkq_chunk(c):
            t = sb_s1.tile((128, WKC), F32, name=r + "wkqc", tag=r + "wkqc", bufs=2)
            nc.scalar.dma_start(t[:], wkqsb_d[:, c * WKC : (c + 1) * WKC])
            wkq_tiles[c] = t

        load_wkq_chunk(0)

        # KQS gemm in fp32: stationary = x^T m-slice, moving = wkqs[j]
        with ExitStack() as pk:
            ps_kq = pk.enter_context(tc.tile_pool(name=r + "pskq", bufs=1, space="PSUM"))
            kq_pair = [
                ps_kq.tile((128, 258), F32, name=f"{r}kqp{i}", tag=f"{r}kqp{i}")
                for i in range(MT // 2)
            ]
            kq_list = [
                kq_pair[t // 2][:, (t % 2) * 129 : (t % 2) * 129 + 129]
                for t in range(MT)
            ]
            for j in range(KD):
                if j in (1, 9, 17):
                    load_wkq_chunk(j // 8 + 1)
                xs_t = sb_s1.tile((128, M), F32, name=r + "xs_t", tag=r + "xs", bufs=4)
                xs_eng = (nc.sync, nc.scalar, nc.gpsimd)[j % 3]
                xs_eng.dma_start(xs_t[:], xt_d[j * 128 : (j + 1) * 128, :])
                nc.scalar.activation(xt_res[j][:], xs_t[:], AF.Copy)
                for t in range(MT):
                    # two m-tiles share one PSUM bank => one accumulation
                    # group: start zeroes the bank at the first sub-tile,
                    # stop closes it at the last
                    nc.tensor.matmul(
                        kq_list[t],
                        xs_t[:, t * 128 : (t + 1) * 128],
                        wkq_tiles[j // 8][:, (j % 8) * 129 : (j % 8 + 1) * 129],
                        start=(j == 0 and t % 2 == 0),
                        stop=(j == KD - 1 and t % 2 == 1),
                    )
            # free all kq PSUM banks promptly: copy u = kq + brow to SBUF
            u_ts = []
            for t in range(MT):
                u_t = sb_s1.tile((128, 129), F32, name=r + "u_t", tag=r + "u", bufs=7)
                nc.vector.tensor_add(u_t[:], kq_list[t], brow_t[:])
                u_ts.append(u_t)

        # per-m-tile phase epilogue (ACT/DVE; overlaps GEMM1 matmuls on PE)
        for t in range(MT):
            u_t = u_ts[t]
            th_t = sb_s1.tile((128, 128), F32, name=r + "th_t", tag=r + "th")
            nc.scalar.activation(th_t[:], u_t[:, 0:128], AF.Tanh)
            d_t = sb_s1.tile((128, 64), F32, name=r + "d_t", tag=r + "d")
            nc.vector.tensor_sub(d_t[:], th_t[:, 0:64], th_t[:, 64:128])
            sn_t = sb_s1.tile((128, 64), F32, name=r + "sn_t", tag=r + "sn")
            nc.scalar.activation(sn_t[:], d_t[:], AF.Sin, scale=PI / 2)
            sq_t = sb_s1.tile((128, 64), F32, name=r + "sq_t", tag=r + "snsq")
            nc.scalar.activation(
                sq_t[:], sn_t[:], AF.Square, accum_out=red_all[:, t : t + 1]
            )
            nc.vector.tensor_scalar(
                align_all[:, t : t + 1],
                red_all[:, t : t + 1],
                -2.0,
                float(P),
                AluOpType.mult,
                AluOpType.add,
            )
            nc.scalar.activation(
                e1_all[:, t : t + 1],
                align_all[:, t : t + 1],
                AF.Exp,
                bias=half_t[:],
                scale=1.0 / P,
            )
            nc.scalar.activation(
                gain_all[:, t : t + 1], e1_all[:, t : t + 1], AF.Ln, bias=1.0
            )
            nc.vector.tensor_mul(
                s2_all[:, t : t + 1],
                align_all[:, t : t + 1],
                gain_all[:, t : t + 1],
            )
            nc.scalar.activation(
                mu_all[:, t : t + 1], u_t[:, 128:129], AF.Copy, scale=1.0 / D
            )

        # GEMM1: w^T tile kd = sum_j Wv^T[j, kd]^T @ x^T[j]  -> SBUF bf16
        # ssq = sum_k w^2 via ones-stationary matmuls, staggered one kd
        # behind the main stream so the PE never waits on sqw.
        ps_v = p1.enter_context(tc.tile_pool(name=r + "psv", bufs=2, space="PSUM"))
        ps_acc = p1.enter_context(tc.tile_pool(name=r + "psacc", bufs=1, space="PSUM"))
        acc_ps0 = ps_acc.tile((1, 512), F32, name=r + "acc_ps0", tag=r + "acc0")
        acc_ps1 = ps_acc.tile((1, 512), F32, name=r + "acc_ps1", tag=r + "acc1")

        sqw_tiles = [None] * KD

        def emit_ssq(kd):
            sqw_t = sqw_tiles[kd]
            nc.tensor.matmul(
                acc_ps0[:], ones_t[:], sqw_t[:, 0:512],
                start=(kd == 0), stop=(kd == KD - 1),
            )
            nc.tensor.matmul(
                acc_ps1[:], ones_t[:], sqw_t[:, 512:1024],
                start=(kd == 0), stop=(kd == KD - 1),
            )

        HD = D // 2
        for kd in range(KD):
            wv_h = []
            for hc in range(2):
                t = sb_s1.tile((128, HD), MMDT, name=r + "wv_t", tag=r + "wv", bufs=3)
                wv_eng = (nc.sync, nc.scalar, nc.gpsimd)[(2 * kd + hc) % 3]
                wv_eng.dma_start(
                    t[:], wvb_d[:, kd * D + hc * HD : kd * D + (hc + 1) * HD]
                )
                wv_h.append(t)
            v_ps = [
                ps_v.tile((128, 512), F32, name=f"{r}v_ps{h}", tag=f"{r}vps{h}")
                for h in range(2)
            ]
            for b in range(KD):
                wv_sl = wv_h[b // 16][:, (b % 16) * 128 : (b % 16 + 1) * 128]
                for h in range(2):
                    nc.tensor.matmul(
                        v_ps[h][:],
                        wv_sl,
                        xt_j(b)[:, h * 512 : (h + 1) * 512],
                        start=(b == 0),
                        stop=(b == KD - 1),
                    )
            if kd >= 1:
                emit_ssq(kd - 1)
            # bias add into fp32 tile; bounce to DRAM for the phase-2
            # residents; bf16 square for the (noise-tolerant) ssq reduction
            wtile = sb_s1.tile((128, M), F32, name=r + "wtile", tag=r + "wtile", bufs=3)
            for h in range(2):
                nc.vector.tensor_scalar(
                    wtile[:, h * 512 : (h + 1) * 512],
                    v_ps[h][:],
                    bvr_t[:, kd : kd + 1],
                    None,
                    AluOpType.add,
                )
            sqw_t = sb_s1.tile((128, M), BF16, name=r + "sqw_t", tag=r + "sqw", bufs=2)
            nc.scalar.activation(sqw_t[:], wtile[:], AF.Square)
            sqw_tiles[kd] = sqw_t
            wb_eng = (nc.gpsimd, nc.sync)[kd % 2]
            wb_eng.dma_start(wt_dram[kd, :, :], wtile[:].bitcast(MMDT))
        emit_ssq(KD - 1)

        # ssq accumulators -> SBUF (cheap engine copies; the DRAM bounce
        # DMAs are deferred into phase 2 so they enqueue BEHIND the
        # wt-reload burst: HWDGE queues are FIFO per engine, and the
        # transposed reads wait on the bounce-write's HBM receipt --
        # emitted here they would block the whole sync queue at the
        # phase boundary)
        nc.scalar.copy(acc_sb[:, 0:512], acc_ps0[:])
        nc.scalar.copy(acc_sb[:, 512:1024], acc_ps1[:])

    # ---------------- phase 2: GEMM2 + epilogue (wt resident) ----------------
    with ExitStack() as p2:
        sb_wt = p2.enter_context(tc.tile_pool(name=r + "wtp", bufs=1))
        sb_s2 = p2.enter_context(tc.tile_pool(name=r + "s2", bufs=2))
        ps_p = p2.enter_context(tc.tile_pool(name=r + "psp", bufs=1, space="PSUM"))

        QN = 4 * 512  # wo chunk: 4 kd-slices of one nb
        NQ = KD // 4  # chunks per nb

        def load_wo_chunk(nb, q, eng):
            t = sb_s2.tile((128, QN), MMDT, name=r + "wo_q", tag=r + "woq", bufs=3)
            base = nb * (KD * 512)
            eng.dma_start(t[:], wob_d[:, base + q * QN : base + (q + 1) * QN])
            return t

        wo_first = load_wo_chunk(0, 0, nc.scalar)

        wt_res = []
        for k in range(KD):
            t = sb_wt.tile((128, M), MMDT, name=f"{r}wtr{k}", tag=f"{r}wtr{k}")
            eng = (nc.gpsimd, nc.sync)[k % 2]
            eng.dma_start(t[:], wt_dram[k, :, :])
            wt_res.append(t)

        w1_res = sb_s2.tile((128, D), F32, name=r + "w1_res", tag=r + "w1_res", bufs=1)
        nc.sync.dma_start(w1_res[:], w1m_d[:, :])

        # deferred ssq bounce: [1, M] -> DRAM -> [128, MT] columns, then
        # the scalar finalize; only gates the first epilogue (~27us in)
        nc.sync.dma_start(ssq_dram[:, :], acc_sb[:])
        for t in range(MT):
            nc.sync.dma_start(
                ssq_all[:, t : t + 1],
                ssq_dram[0:1, t * 128 : (t + 1) * 128].transpose([1, 0]),
            )
        nc.scalar.activation(musq_all[:], mu_all[:], AF.Square)
        nc.vector.tensor_scalar(var_all[:], ssq_all[:], 1.0 / D, None, AluOpType.mult)
        nc.vector.tensor_sub(var_all[:], var_all[:], musq_all[:])
        nc.scalar.activation(s_all[:], s2_all[:], AF.Copy, scale=1.0 / P)
        nc.scalar.activation(s_sq_all[:], s_all[:], AF.Square)
        nc.vector.tensor_mul(q_all[:], var_all[:], s_sq_all[:])
        nc.scalar.activation(q2_all[:], q_all[:], AF.Sqrt, bias=eps_t[:])
        nc.vector.reciprocal(inv_all[:], q2_all[:])
        nc.vector.tensor_mul(a_all[:], s_all[:], inv_all[:])
        nc.vector.tensor_mul(c_all[:], a_all[:], mu_all[:])
        nc.vector.tensor_scalar(cneg_all[:], c_all[:], -1.0, None, AluOpType.mult)

        def epilogue(nb, mt, p_tile):
            nsl = slice(nb * 512, (nb + 1) * 512)
            msl = slice(mt * 128, (mt + 1) * 128)
            xe_t = sb_s2.tile((128, 512), F32, name=r + "xe_t", tag=r + "xe", bufs=3)
            nc.gpsimd.dma_start(xe_t[:], xrm_d[msl, nsl])
            t1_t = sb_s2.tile((128, 512), F32, name=r + "t1_t", tag=r + "t1", bufs=3)
            nc.vector.scalar_tensor_tensor(
                t1_t[:], p_tile[:], a_all[:, mt : mt + 1], xe_t[:],
                AluOpType.mult, AluOpType.add,
            )
            oe_t = sb_s2.tile((128, 512), F32, name=r + "oe_t", tag=r + "oe", bufs=4)
            nc.vector.scalar_tensor_tensor(
                oe_t[:], w1_res[:, nsl], cneg_all[:, mt : mt + 1], t1_t[:],
                AluOpType.mult, AluOpType.add,
            )
            nc.sync.dma_start(out_d[msl, nsl], oe_t[:])

        for nb in range(NB):
            wo_q = []
            for q in range(NQ):
                if nb == 0 and q == 0:
                    wo_q.append(wo_first)
                    continue
                wo_q.append(load_wo_chunk(nb, q, nc.scalar))

            p_tiles = [
                ps_p.tile((128, 512), F32, name=f"{r}pp{mt}", tag=f"{r}pp{mt}")
                for mt in range(MT)
            ]
            for half in range(2):
                mts = range(half * 4, half * 4 + 4)
                for kd in range(KD):
                    wo_sl = wo_q[kd // 4][:, (kd % 4) * 512 : (kd % 4 + 1) * 512]
                    for mt in mts:
                        nc.tensor.matmul(
                            p_tiles[mt][:],
                            wt_res[kd][:, mt * 128 : (mt + 1) * 128],
                            wo_sl,
                            start=(kd == 0),
                            stop=(kd == KD - 1),
                        )
                for mt in mts:
                    epilogue(nb, mt, p_tiles[mt])


def build_nc(reps=1):
    nc = bass.Bass()
    xt_d = nc.declare_dram_parameter("xt", [D, M], F32, isOutput=False)
    wvb_d = nc.declare_dram_parameter("wvb", [128, KD * D], MMDT, isOutput=False)
    wob_d = nc.declare_dram_parameter("wob", [128, NB * KD * 512], MMDT, isOutput=False)
    wkqsb_d = nc.declare_dram_parameter("wkqsb", [128, KD * 129], F32, isOutput=False)
    brow_d = nc.declare_dram_parameter("brow", [128, 129], F32, isOutput=False)
    bvr_d = nc.declare_dram_parameter("bvr", [128, KD], F32, isOutput=False)
    w1m_d = nc.declare_dram_parameter("w1m", [128, D], F32, isOutput=False)
    xrm_d = nc.declare_dram_parameter("xrm", [M, D], F32, isOutput=False)
    out_d = nc.declare_dram_parameter("out", [M, D], F32, isOutput=True)
    ssq_dram = nc.dram_tensor("ssq_scr", [1, M], F32)
    wt_dram = nc.dram_tensor("wt_scr", [KD, 128, M], MMDT)
    dram = (xt_d, wvb_d, wob_d, wkqsb_d, brow_d, bvr_d, w1m_d, xrm_d,
            out_d, ssq_dram, wt_dram)

    for rep in range(reps):
        with ExitStack() as ctx:
            tc = ctx.enter_context(LegalTileContext(nc))
            build_body(nc, tc, ctx, dram, rep)
    return nc


def prep_host(inputs):
    """Host-side weight layout prep shared by all cores."""
    Wk = np.asarray(inputs["Wk"], dtype=np.float32)
    bk = np.asarray(inputs["bk"], dtype=np.float32)
    Wq = np.asarray(inputs["Wq"], dtype=np.float32)
    bq = np.asarray(inputs["bq"], dtype=np.float32)
    Wv = np.asarray(inputs["Wv"], dtype=np.float32)
    bv = np.asarray(inputs["bv"], dtype=np.float32)
    ln_g = np.asarray(inputs["ln_g"], dtype=np.float32)
    ln_b = np.asarray(inputs["ln_b"], dtype=np.float32)
    Wo = np.asarray(inputs["Wo"], dtype=np.float32)
    bo = np.asarray(inputs["bo"], dtype=np.float32)

    Wo2T = np.ascontiguousarray((Wo * ln_g[None, :]).T)  # [k, n]
    w1 = Wo2T.sum(axis=0)  # [n]
    rrow = (ln_b @ Wo.T + bo).astype(np.float32)  # [n]
    wsum = Wv.sum(axis=0)  # [j]
    wkqs = np.concatenate([Wk.T, Wq.T, wsum[:, None]], axis=1).astype(np.float32)
    brow = np.concatenate([bk, bq, [bv.sum()]]).astype(np.float32)

    # [128, KD*D]: wvb[p, kd*D + b*128 + kk] = Wv[kd*128+kk, b*128+p]
    wvb = np.ascontiguousarray(
        Wv.reshape(KD, 128, KD, 128).transpose(3, 0, 2, 1).reshape(128, KD * D)
    )
    # [128, NB*KD*512]: wob[p, nb*KD*512 + kd*512 + n'] = Wo2T[kd*128+p, nb*512+n']
    wob = np.ascontiguousarray(
        Wo2T.reshape(KD, 128, NB, 512).transpose(1, 2, 0, 3).reshape(128, NB * KD * 512)
    )
    # [128, KD*129]: wkqsb[p, j*129+c] = wkqs[j*128+p, c]  (fp32)
    wkqsb = np.ascontiguousarray(
        wkqs.reshape(KD, 128, 129).transpose(1, 0, 2).reshape(128, KD * 129)
    )
    brow_mat = np.ascontiguousarray(np.broadcast_to(brow, (128, 129)))
    bvr = np.ascontiguousarray(bv.reshape(KD, 128).T)  # [128, KD]
    w1m = np.ascontiguousarray(np.broadcast_to(w1, (128, D)), dtype=np.float32)
    return wvb, wob, wkqsb, brow_mat, bvr, w1m, rrow


def make_in_maps(x, prep):
    wvb, wob, wkqsb, brow_mat, bvr, w1m, rrow = prep
    in_maps = []
    for c in range(NCORES):
        xc = np.ascontiguousarray(x[c * M : (c + 1) * M])
        xtc = np.ascontiguousarray(xc.T)
        xrm = np.ascontiguousarray(xc + rrow[None, :])
        in_maps.append(
            {
                "xt": xtc,
                "wvb": wvb,
                "wob": wob,
                "wkqsb": wkqsb,
                "brow": brow_mat,
                "bvr": bvr,
                "w1m": w1m,
                "xrm": xrm,
            }
        )
    return in_maps


def kernel(**inputs):
    x = np.asarray(inputs["x"], dtype=np.float32)
    prep = prep_host(inputs)
    nc = build_nc(reps=1)
    in_maps = make_in_maps(x, prep)
    global LAST_BUILD
    LAST_BUILD = (nc, in_maps)
    res = run_bass_kernel_spmd(nc, in_maps, core_ids=list(range(NCORES)))
    global LAST_EXEC_NS
    LAST_EXEC_NS = getattr(res, "exec_time_ns", None)
    out = np.concatenate([res.results[c]["out"] for c in range(NCORES)], axis=0)
    return out.astype(np.float32)


LAST_EXEC_NS = None
LAST_BUILD = None


if __name__ == "__main__":
    rng = np.random.default_rng(0)
    ins = {
        "x": rng.standard_normal((B, D), dtype=np.float32),
        "Wk": rng.standard_normal((P, D), dtype=np.float32) / math.sqrt(D),
        "bk": np.zeros(P, np.float32),
        "Wq": rng.standard_normal((P, D), dtype=np.float32) / math.sqrt(D),
        "bq": np.zeros(P, np.float32),
        "Wv": rng.standard_normal((D, D), dtype=np.float32) / math.sqrt(D),
        "bv": np.zeros(D, np.float32),
        "ln_g": np.ones(D, np.float32),
        "ln_b": np.zeros(D, np.float32),
        "Wo": rng.standard_normal((D, D), dtype=np.float32) / math.sqrt(D),
        "bo": np.zeros(D, np.float32),
    }
    out = kernel(**ins)
    print("out", out.shape, out.dtype, float(np.abs(out).mean()))
